# revision 19
# baseline (speedup 1.0000x reference)
"""Trainium2 Bass kernel for nn_ClassifyingReconstructionLoss.

loss = (1/B) * sum_{n,b} p[n,b] * (logsumexp(y_pred[n,b,:]) - y_pred[n,b,y_true[b]-1])

Sharding: step-parallel across the 8 NeuronCores (n = 8 steps, one per core).
Each core streams its (128 batch x 32000 vocab) shard from HBM as fp8-e4m3
(host-side downcast: quarter of the f32 HBM traffic) and computes per-row
sum(exp(x)) using THREE engines in parallel, split by vocab columns:

  - ACT (scalar engine, 1.2 GHz, 1 elem/cycle): exact table exp with the
    free fused accumulator (accum_out) on ~53% of columns.
  - GPSIMD (pool engine, ~0.86 ns/elem): Schraudolph exp on the other ~47%:
    i16 = round(x * 184.665 + 16248.5); those int16 bits, reinterpreted as
    bfloat16, equal exp(x) within +-4% per element (the magic constant is
    tuned so the exp-weighted mean error on the fp8 grid is ~0, so row sums
    are accurate to ~2e-3).
  - DVE (vector engine): sums the Schraudolph bits via tensor_scalar's
    fused accumulator over the bf16 bit-view (1 elem/cycle, its fastest
    reduce mode).

The tiny per-row log / gather / p-weighted reduction (8*128 elements) is
done on the host, as is the final scalar all-reduce across cores.

Raw Bass (explicit semaphores): the TileContext scheduler emits instructions
with >1 sync wait, which this walrus rejects ("Too many sync wait commands").
"""

import contextlib
import sys

import ml_dtypes
import numpy as np

sys.path.insert(0, "/opt/trn_rl_repo")

import concourse.bass as bass
import concourse.mybir as mybir
from concourse.bass_utils import run_bass_kernel_spmd

N_STEPS, BATCH, VOCAB = 8, 128, 32000
N_CORES = 8

# Schraudolph constants for bf16-bit exp: i16 = round(x*A_S + B_S);
# bitcast bf16 ~= exp(x). B_S tuned for zero exp-weighted mean error on the
# e4m3 input grid (see docstring).
A_S = 184.6649652337873  # 128 * log2(e)
B_S = 16248.5015

# Vocab chunk ladder (sum = VOCAB): small first chunks start the engines
# early while later chunks stream (the 16 SDMA engines interleave all queued
# transfers, so chunk j's completion lands at roughly the cumulative-bytes
# fraction of the ~11.3us DMA window); big later chunks amortize
# per-instruction overhead (ACT pays ~220ns/instr, POOL ~150, DVE ~90).
CHUNKS = [1680, 3072, 3352, 4824, 5216, 5856, 8000]
# Per-chunk column split: ACT gets the first CHUNKS-CU columns (exact exp),
# the CU go to GPSIMD-convert + DVE-accumulate. Tuned so ACT (0.883 ns/elem)
# and the DVE accumulate (1.066 ns/elem) finish together given the chunk
# release times. 2-port DVE ops contend with GPSIMD's shared SBUF port and
# drop below 1x (measured), so pairwise tensor_tensor halving (TT, 0.52
# ns/elem at 2x) is only used where the pool is already done: the last chunk.
CU = [704, 1880, 1760, 2528, 1472, 2520, 4008]
TT = [False, False, False, False, False, False, True]
CA = [c - u for c, u in zip(CHUNKS, CU)]
NCH = len(CHUNKS)

_cached_nc = None


def build_nc():
    f32 = mybir.dt.float32
    bf16 = mybir.dt.bfloat16
    i16 = mybir.dt.int16
    fp8 = mybir.dt.float8e4
    alu = mybir.AluOpType

    offs = [sum(CHUNKS[:j]) for j in range(NCH)]
    uoffs = [sum(CU[:j]) for j in range(NCH)]
    u_total = sum(CU)
    max_ca = max(CA)

    nc = bass.Bass(trn_type="TRN2")
    x = nc.declare_dram_parameter("x", [BATCH, VOCAB], fp8, isOutput=False)
    sa = nc.declare_dram_parameter("sums_act", [BATCH, NCH], f32, isOutput=True)
    sd = nc.declare_dram_parameter("sums_dve", [BATCH, NCH], f32, isOutput=True)

    with (
        nc.sbuf_tensor([BATCH, VOCAB], fp8) as xin,
        nc.sbuf_tensor([BATCH, u_total], i16) as conv,
        nc.sbuf_tensor([BATCH, max_ca], bf16) as aout,
        nc.sbuf_tensor([BATCH, NCH], f32) as sa_t,
        nc.sbuf_tensor([BATCH, NCH], f32) as sd_t,
        nc.Block(no_gpsimd_drain=True) as block,
    ):
        with contextlib.ExitStack() as st:
            chunk_sems = [
                st.enter_context(nc.semaphore(f"ch_sem{j}")) for j in range(NCH)
            ]
            pool_sem = st.enter_context(nc.semaphore("pool_sem"))
            out_sem = st.enter_context(nc.semaphore("out_sem"))
            act_sem = st.enter_context(nc.semaphore("act_sem"))
            dve_done = st.enter_context(nc.semaphore("dve_done"))
            warm = st.enter_context(nc.sbuf_tensor([BATCH, 1], f32))
            zbias = st.enter_context(nc.sbuf_tensor([BATCH, 1], f32))

            @block.sync
            def _(sync):
                for j in range(NCH):
                    sync.dma_start(
                        out=xin[:, offs[j] : offs[j] + CHUNKS[j]],
                        in_=x[:, offs[j] : offs[j] + CHUNKS[j]],
                    ).then_inc(chunk_sems[j], 16)
                sync.wait_ge(dve_done, 1)
                sync.dma_start(out=sd[:], in_=sd_t[:]).then_inc(out_sem, 16)
                sync.wait_ge(out_sem, 32)

            @block.scalar
            def _(scalar):
                # ACT zeroes its own bias tile (keeps const-AP memsets out of
                # the preamble); the dummy 1-col exp pulls the ~1.3us
                # ACT_TABLE_LOAD off the critical path (overlaps chunk 0 DMA).
                nc.scalar.memzero(zbias.ap()).then_inc(act_sem, 1)
                scalar.wait_ge(act_sem, 1)
                nc.scalar.activation(
                    warm.ap(), zbias.ap(), mybir.ActivationFunctionType.Exp,
                    bias=zbias.ap(),
                )
                for j in range(NCH):
                    scalar.wait_ge(chunk_sems[j], 16)
                    nc.scalar.activation(
                        aout[:, : CA[j]],
                        xin[:, offs[j] : offs[j] + CA[j]],
                        mybir.ActivationFunctionType.Exp,
                        bias=zbias.ap(),
                        accum_out=sa_t[:, j : j + 1],
                    ).then_inc(act_sem, 1)
                # the sequencer runs ahead of the engine: without this wait the
                # HWDGE DMA would ship sa_t before the last accums land
                scalar.wait_ge(act_sem, NCH + 1)
                scalar.dma_start(out=sa[:], in_=sa_t[:]).then_inc(out_sem, 16)

            @block.gpsimd
            def _(gpsimd):
                for j in range(NCH):
                    gpsimd.wait_ge(chunk_sems[j], 16)
                    nc.gpsimd.tensor_scalar(
                        conv[:, uoffs[j] : uoffs[j] + CU[j]],
                        xin[:, offs[j] + CA[j] : offs[j] + CHUNKS[j]],
                        A_S, B_S, alu.mult, alu.add,
                    ).then_inc(pool_sem, 1)

            @block.vector
            def _(vector):
                for j in range(NCH):
                    vector.wait_ge(pool_sem, j + 1)
                    cv = conv[:, uoffs[j] : uoffs[j] + CU[j]].bitcast(bf16)
                    if TT[j]:
                        h, q = CU[j] // 2, CU[j] // 4
                        nc.vector.tensor_tensor(
                            cv[:, :h], cv[:, :h], cv[:, h:], alu.add
                        )
                        nc.vector.tensor_tensor(
                            cv[:, :q], cv[:, :q], cv[:, q:h], alu.add
                        )
                        cv = cv[:, :q]
                    ins = nc.vector.tensor_scalar(
                        cv, cv, 1.0, 0.0, alu.mult, alu.add,
                        accum_out=sd_t[:, j : j + 1],
                    )
                    if j == NCH - 1:
                        ins.then_inc(dve_done, 1)

    # Strip the framework preamble this kernel no longer depends on (const-AP
    # memsets + entry drains/event-sems): nothing here reads const APs.
    blk = nc.m.functions[0].blocks[0]
    blk.instructions[:] = [
        i
        for i in blk.instructions
        if type(i).__name__ not in ("InstMemset", "InstDrain", "InstEventSemaphore")
    ]
    return nc


def kernel(p, y_pred, y_true, pad_id):
    global _cached_nc
    p = np.asarray(p)
    y_pred = np.asarray(y_pred)
    y_true = np.asarray(y_true)
    if _cached_nc is None:
        _cached_nc = build_nc()

    in_maps = [
        {"x": y_pred[c].astype(ml_dtypes.float8_e4m3)} for c in range(N_CORES)
    ]
    res = run_bass_kernel_spmd(_cached_nc, in_maps, list(range(N_CORES)))
    s_act = np.stack(
        [res.results[i]["sums_act"] for i in range(N_CORES)]
    )  # (n, B, NCH)
    s_dve = np.stack([res.results[i]["sums_dve"] for i in range(N_CORES)])

    sums = s_act.astype(np.float64).sum(axis=-1) + s_dve.astype(np.float64).sum(
        axis=-1
    )  # (n, B)
    lse = np.log(sums)
    idx = y_true.astype(np.int64) - 1
    gathered = y_pred[:, np.arange(BATCH), idx]  # (n, B)
    loss = (p.astype(np.float64) * (lse - gathered)).sum() / BATCH
    return np.float32(loss)


# revision 20
# speedup vs baseline: 1.0188x; 1.0188x over previous
"""Trainium2 Bass kernel for nn_ClassifyingReconstructionLoss.

loss = (1/B) * sum_{n,b} p[n,b] * (logsumexp(y_pred[n,b,:]) - y_pred[n,b,y_true[b]-1])

Sharding: step-parallel across the 8 NeuronCores (n = 8 steps, one per core).
Each core streams its (128 batch x 32000 vocab) shard from HBM as fp8-e4m3
(host-side downcast: quarter of the f32 HBM traffic) and computes per-row
sum(exp(x)) using THREE engines in parallel, split by vocab columns:

  - ACT (scalar engine, 1.2 GHz, 1 elem/cycle): exact table exp with the
    free fused accumulator (accum_out) on ~53% of columns.
  - GPSIMD (pool engine, ~0.86 ns/elem): Schraudolph exp on the other ~47%:
    i16 = round(x * 184.665 + 16248.5); those int16 bits, reinterpreted as
    bfloat16, equal exp(x) within +-4% per element (the magic constant is
    tuned so the exp-weighted mean error on the fp8 grid is ~0, so row sums
    are accurate to ~2e-3).
  - DVE (vector engine): sums the Schraudolph bits via tensor_scalar's
    fused accumulator over the bf16 bit-view (1 elem/cycle, its fastest
    reduce mode).

The tiny per-row log / gather / p-weighted reduction (8*128 elements) is
done on the host, as is the final scalar all-reduce across cores.

Raw Bass (explicit semaphores): the TileContext scheduler emits instructions
with >1 sync wait, which this walrus rejects ("Too many sync wait commands").
"""

import contextlib
import sys

import ml_dtypes
import numpy as np

sys.path.insert(0, "/opt/trn_rl_repo")

import concourse.bass as bass
import concourse.mybir as mybir
from concourse.bass_utils import run_bass_kernel_spmd

N_STEPS, BATCH, VOCAB = 8, 128, 32000
N_CORES = 8

# Schraudolph constants for bf16-bit exp: i16 = round(x*A_S + B_S);
# bitcast bf16 ~= exp(x). B_S tuned for zero exp-weighted mean error on the
# e4m3 input grid (see docstring).
A_S = 184.6649652337873  # 128 * log2(e)
B_S = 16248.5015

# Vocab chunk ladder (sum = VOCAB): small first chunks start the engines
# early while later chunks stream (the 16 SDMA engines interleave all queued
# transfers, so chunk j's completion lands at roughly the cumulative-bytes
# fraction of the ~11.3us DMA window); big later chunks amortize
# per-instruction overhead (ACT pays ~220ns/instr, POOL ~150, DVE ~90).
CHUNKS = [1680, 3072, 3352, 5424, 4616, 5856, 8000]
# Per-chunk column split: ACT gets the first CHUNKS-CU columns (exact exp),
# the CU go to GPSIMD-convert + DVE-accumulate. Tuned so ACT (0.883 ns/elem)
# and the DVE accumulate (1.066 ns/elem) finish together given the chunk
# release times. 2-port DVE ops contend with GPSIMD's shared SBUF port and
# drop below 1x (measured), so pairwise tensor_tensor halving (TT, 0.52
# ns/elem at 2x) is only used where the pool is already done: the last chunk.
CU = [704, 1880, 1760, 2840, 1128, 2240, 4008]
TT = [False, False, False, False, False, False, True]
CA = [c - u for c, u in zip(CHUNKS, CU)]
NCH = len(CHUNKS)

_cached_nc = None


def build_nc():
    f32 = mybir.dt.float32
    bf16 = mybir.dt.bfloat16
    i16 = mybir.dt.int16
    fp8 = mybir.dt.float8e4
    alu = mybir.AluOpType

    offs = [sum(CHUNKS[:j]) for j in range(NCH)]
    uoffs = [sum(CU[:j]) for j in range(NCH)]
    u_total = sum(CU)
    max_ca = max(CA)

    nc = bass.Bass(trn_type="TRN2")
    x = nc.declare_dram_parameter("x", [BATCH, VOCAB], fp8, isOutput=False)
    sa = nc.declare_dram_parameter("sums_act", [BATCH, NCH], f32, isOutput=True)
    sd = nc.declare_dram_parameter("sums_dve", [BATCH, NCH], f32, isOutput=True)

    with (
        nc.sbuf_tensor([BATCH, VOCAB], fp8) as xin,
        nc.sbuf_tensor([BATCH, u_total], i16) as conv,
        nc.sbuf_tensor([BATCH, max_ca], bf16) as aout,
        nc.sbuf_tensor([BATCH, NCH], f32) as sa_t,
        nc.sbuf_tensor([BATCH, NCH], f32) as sd_t,
        nc.Block(no_gpsimd_drain=True) as block,
    ):
        with contextlib.ExitStack() as st:
            chunk_sems = [
                st.enter_context(nc.semaphore(f"ch_sem{j}")) for j in range(NCH)
            ]
            pool_sem = st.enter_context(nc.semaphore("pool_sem"))
            out_sem = st.enter_context(nc.semaphore("out_sem"))
            act_sem = st.enter_context(nc.semaphore("act_sem"))
            dve_done = st.enter_context(nc.semaphore("dve_done"))
            warm = st.enter_context(nc.sbuf_tensor([BATCH, 1], f32))
            zbias = st.enter_context(nc.sbuf_tensor([BATCH, 1], f32))

            @block.sync
            def _(sync):
                for j in range(NCH):
                    sync.dma_start(
                        out=xin[:, offs[j] : offs[j] + CHUNKS[j]],
                        in_=x[:, offs[j] : offs[j] + CHUNKS[j]],
                    ).then_inc(chunk_sems[j], 16)
                sync.wait_ge(dve_done, 1)
                sync.dma_start(out=sd[:], in_=sd_t[:]).then_inc(out_sem, 16)
                sync.wait_ge(out_sem, 32)

            @block.scalar
            def _(scalar):
                # ACT zeroes its own bias tile (keeps const-AP memsets out of
                # the preamble); the dummy 1-col exp pulls the ~1.3us
                # ACT_TABLE_LOAD off the critical path (overlaps chunk 0 DMA).
                nc.scalar.memzero(zbias.ap()).then_inc(act_sem, 1)
                scalar.wait_ge(act_sem, 1)
                nc.scalar.activation(
                    warm.ap(), zbias.ap(), mybir.ActivationFunctionType.Exp,
                    bias=zbias.ap(),
                )
                for j in range(NCH):
                    scalar.wait_ge(chunk_sems[j], 16)
                    nc.scalar.activation(
                        aout[:, : CA[j]],
                        xin[:, offs[j] : offs[j] + CA[j]],
                        mybir.ActivationFunctionType.Exp,
                        bias=zbias.ap(),
                        accum_out=sa_t[:, j : j + 1],
                    ).then_inc(act_sem, 1)
                # the sequencer runs ahead of the engine: without this wait the
                # HWDGE DMA would ship sa_t before the last accums land
                scalar.wait_ge(act_sem, NCH + 1)
                scalar.dma_start(out=sa[:], in_=sa_t[:]).then_inc(out_sem, 16)

            @block.gpsimd
            def _(gpsimd):
                for j in range(NCH):
                    gpsimd.wait_ge(chunk_sems[j], 16)
                    nc.gpsimd.tensor_scalar(
                        conv[:, uoffs[j] : uoffs[j] + CU[j]],
                        xin[:, offs[j] + CA[j] : offs[j] + CHUNKS[j]],
                        A_S, B_S, alu.mult, alu.add,
                    ).then_inc(pool_sem, 1)

            @block.vector
            def _(vector):
                for j in range(NCH):
                    vector.wait_ge(pool_sem, j + 1)
                    cv = conv[:, uoffs[j] : uoffs[j] + CU[j]].bitcast(bf16)
                    if TT[j]:
                        h, q = CU[j] // 2, CU[j] // 4
                        nc.vector.tensor_tensor(
                            cv[:, :h], cv[:, :h], cv[:, h:], alu.add
                        )
                        nc.vector.tensor_tensor(
                            cv[:, :q], cv[:, :q], cv[:, q:h], alu.add
                        )
                        cv = cv[:, :q]
                    ins = nc.vector.tensor_scalar(
                        cv, cv, 1.0, 0.0, alu.mult, alu.add,
                        accum_out=sd_t[:, j : j + 1],
                    )
                    if j == NCH - 1:
                        ins.then_inc(dve_done, 1)

    # Strip the framework preamble this kernel no longer depends on (const-AP
    # memsets + entry drains/event-sems): nothing here reads const APs.
    blk = nc.m.functions[0].blocks[0]
    blk.instructions[:] = [
        i
        for i in blk.instructions
        if type(i).__name__ not in ("InstMemset", "InstDrain", "InstEventSemaphore")
    ]
    return nc


def kernel(p, y_pred, y_true, pad_id):
    global _cached_nc
    p = np.asarray(p)
    y_pred = np.asarray(y_pred)
    y_true = np.asarray(y_true)
    if _cached_nc is None:
        _cached_nc = build_nc()

    in_maps = [
        {"x": y_pred[c].astype(ml_dtypes.float8_e4m3)} for c in range(N_CORES)
    ]
    res = run_bass_kernel_spmd(_cached_nc, in_maps, list(range(N_CORES)))
    s_act = np.stack(
        [res.results[i]["sums_act"] for i in range(N_CORES)]
    )  # (n, B, NCH)
    s_dve = np.stack([res.results[i]["sums_dve"] for i in range(N_CORES)])

    sums = s_act.astype(np.float64).sum(axis=-1) + s_dve.astype(np.float64).sum(
        axis=-1
    )  # (n, B)
    lse = np.log(sums)
    idx = y_true.astype(np.int64) - 1
    gathered = y_pred[:, np.arange(BATCH), idx]  # (n, B)
    loss = (p.astype(np.float64) * (lse - gathered)).sum() / BATCH
    return np.float32(loss)
